# revision 24
# baseline (speedup 1.0000x reference)
"""GCN (3-layer, PyG GCNConv semantics) on 8 Trainium2 NeuronCores.

Strategy (graph/data parallel, dst-sharded), v2:
  - Nodes sharded across 8 cores (rows of x / output).
  - Per layer: each core computes its slice of h = y_prev @ W on PE,
    writes it (fp16, 256B-strided rows) to DRAM, AllGather -> full g table.
  - Aggregation: edges bucketed by (dst tile, src class) where class is
    "local" (src in own shard, gathered from the pre-collective shard_d,
    overlapping the AllGather) or one of ceil(N/32768) regions of gfull
    (int16 gather indices are region-relative).  Chunks of 128 edges;
    dma_gather (custom 128B-payload lowering) fetches g[src] rows across
    4 concurrent SWDGE queues (num_swdge_queues=4, queue_num rotation —
    ~2.2 ns/row vs 8 single-queue); a norm-valued one-hot S
    ([128 edges x 128 dst], ONE dual-op tensor_scalar: (iota==dst)*norm)
    turns the scatter-add into PE matmuls accumulated in PSUM:
    agg_T = G.T @ S.  Self-loops are a diag chunk (dst=iota, norm=invdeg)
    whose lhsT is the local activation tile (no gather).
  - Epilogue: relu(agg + b) in one ScalarE activation (transposed layout:
    bias is per-partition). Final layer: log_softmax via exp (ACT),
    partition-sum (PE ones-matmul), ln (ACT), broadcast (PE), subtract.
  - Output is produced transposed [40, nodes] per core; host transposes.

Self-contained: only needs numpy + the concourse stack at /opt/trn_rl_repo.
"""

import os
import sys

sys.path.insert(0, "/opt/trn_rl_repo")

import numpy as np

import concourse.bacc as bacc
import concourse.tile as tile
import concourse.mybir as mybir
from concourse import ap_utils
from concourse.bass import AP, MemorySpace
from concourse.bass_utils import run_bass_kernel_spmd

fp32 = mybir.dt.float32
fp16 = mybir.dt.float16
i16 = mybir.dt.int16

N_CORES = 8
P = 128
NBMAX = 8          # gather blocks per call (single-packet limit: 1024 rows)
SL_CAP = 4095      # max slice rows: N_CORES*SL must fit int16 gather index
NQ = 4             # SWDGE queues

# test.py sets this to capture profile info; harness leaves it off.
TRACE = os.environ.get("GCN_TRACE", "0") == "1"
# strided (64-col) collective APs are rejected by walrus; keep full rows
AG64 = os.environ.get("GCN_AG64", "0") == "1"
LAST_RESULT = None


# ---------------------------------------------------------------- gather ----
def dma_gather_raw(engine, out_ap, in_ap, idxs_ap, num_idxs, elem_size, elem_step,
                   queue_num=0, single_packet=True):
    """bass dma_gather minus the elem_size%256B assert: the ucode only needs
    the row STRIDE 256B-quantized; the payload is free."""
    assert idxs_ap.dtype == mybir.dt.int16
    assert in_ap.space == MemorySpace.DRAM
    assert out_ap.space == MemorySpace.SBUF
    assert in_ap.dtype == out_ap.dtype
    assert ap_utils.ap_is_contiguous(out_ap.ap[1:])
    assert ap_utils.ap_is_contiguous(idxs_ap.ap[1:])
    assert in_ap.ap[-1][1] == elem_size
    assert out_ap.ap[-1][1] == elem_size
    assert in_ap.ap[0][0] == elem_step
    stride_bytes = elem_step * mybir.dt.size(in_ap.dtype)
    assert stride_bytes % 256 == 0
    return engine.add_instruction(
        mybir.InstDMAGatherAnt(
            name=engine.bass.get_next_instruction_name(),
            ins=[
                *engine.lower_ap_dma(in_ap, for_custom_bir_dma=True),
                engine.lower_ap(idxs_ap),
                engine.lower_val_access(engine.to_reg(num_idxs)),
            ],
            outs=[engine.lower_ap(out_ap)],
            transpose=False,
            num_idxs=num_idxs,
            elem_size=elem_size,
            stride_bytes_256=stride_bytes // 256,
            gen_mode=0,
            single_packet=single_packet,
            queue_num=queue_num,
            sbuf_tokens_per_rank=0,
            sbuf_free_dim_per_rank=0,
            sbuf_free_dim_pad_per_rank=0,
            sbuf_byte_offset=0,
        )
    )


# ---------------------------------------------------------- host preprocess --
def _prepare_spmd(edge_index, n, npc):
    """Shared (max-over-cores) chunk layout so all cores run one program.

    Src classes: 0 = local (own shard, gathered from shard_d pre-collective),
    1+k = slice k.  The AllGather is split into NR sub-collectives; sub-AG k
    gathers every core's local rows [k*SL, (k+1)*SL) into a contiguous
    gfull region of N_CORES*SL rows ordered (core, row%SL), so slice-k
    gathers can start as soon as sub-AG k lands.
    """
    src = np.asarray(edge_index[0], np.int64)
    dst = np.asarray(edge_index[1], np.int64)
    deg = np.bincount(dst, minlength=n).astype(np.float64) + 1.0
    dis = 1.0 / np.sqrt(deg)
    norm_all = (dis[src] * dis[dst]).astype(np.float32)
    invdeg = (1.0 / deg).astype(np.float32)

    NR = (npc + SL_CAP - 1) // SL_CAP
    while npc % NR != 0 and NR < npc:
        NR += 1
    SL = npc // NR
    n_tiles = (npc + P - 1) // P
    NCLS = 1 + NR

    # per-core bucket counts
    cnts = np.zeros((N_CORES, n_tiles, NCLS), np.int64)
    core_edges = []
    for c in range(N_CORES):
        m = (dst >= c * npc) & (dst < (c + 1) * npc)
        s_c, d_c, nrm_c = src[m], dst[m] - c * npc, norm_all[m]
        tile_c = d_c // P
        own = (s_c >= c * npc) & (s_c < (c + 1) * npc)
        src_core = s_c // npc
        src_loc = s_c - src_core * npc
        slc = src_loc // SL
        cls = np.where(own, 0, 1 + slc)
        rel = np.where(own, src_loc,
                       src_core * SL + (src_loc - slc * SL))
        order = np.lexsort((s_c, cls, tile_c))
        tile_s, cls_s = tile_c[order], cls[order]
        bucket = tile_s * NCLS + cls_s
        cnts[c] = np.bincount(bucket, minlength=n_tiles * NCLS).reshape(
            n_tiles, NCLS)
        core_edges.append((bucket, rel[order], (d_c[order] - tile_s * P),
                           nrm_c[order]))

    kc = (cnts.max(axis=0) + P - 1) // P  # shared chunks per (tile, cls)

    ranges = []
    start = 0
    while start < n_tiles:
        end = start + 1
        while end < n_tiles:
            if kc[start:end + 1].sum(axis=0).max() > NBMAX:
                break
            end += 1
        ranges.append((start, end))
        start = end

    chunk_base = np.zeros((n_tiles, NCLS), np.int64)
    calls = []
    g = 0
    for (t0, t1) in ranges:
        for q in range(NCLS):
            nb = int(kc[t0:t1, q].sum())
            if nb == 0:
                continue
            lo = g
            for t in range(t0, t1):
                chunk_base[t, q] = g
                g += int(kc[t, q])
            # split oversized calls (single tile can exceed NBMAX chunks):
            # the 1024-row single-packet limit caps nb at 8 per call
            while nb > NBMAX:
                calls.append((q, lo, NBMAX))
                lo += NBMAX
                nb -= NBMAX
            calls.append((q, lo, nb))
    nchunk_g = g
    chunks_of_tile = []
    for t in range(n_tiles):
        lst = []
        for q in range(NCLS):
            for k in range(int(kc[t, q])):
                lst.append((int(chunk_base[t, q]) + k, q))
        chunks_of_tile.append(lst)

    nchunk_all = nchunk_g + n_tiles
    per_core = []
    for c in range(N_CORES):
        bucket, rel_s, dl_s, nrm_s = core_edges[c]
        cnt = cnts[c].reshape(-1)
        bs = np.concatenate([[0], np.cumsum(cnt)[:-1]])
        rank = np.arange(len(bucket)) - bs[bucket]
        t_of = bucket // NCLS
        q_of = bucket % NCLS
        gchunk = chunk_base[t_of, q_of] + rank // P
        slot = rank % P

        idx_flat = np.zeros(max(nchunk_g, 1) * P, np.int64)
        idx_flat[gchunk * P + slot] = rel_s
        dstl = np.zeros((P, nchunk_all), np.float32)
        nrmv = np.zeros((P, nchunk_all), np.float32)
        dstl[slot, gchunk] = dl_s
        nrmv[slot, gchunk] = nrm_s
        node = c * npc + np.arange(npc)
        dstl[np.arange(npc) % P, nchunk_g + np.arange(npc) // P] = \
            np.arange(npc) % P
        nrmv[np.arange(npc) % P, nchunk_g + np.arange(npc) // P] = invdeg[node]

        tmp = idx_flat.astype(np.int16).reshape(max(nchunk_g, 1) * 8, 16).T
        idx16 = np.tile(np.ascontiguousarray(tmp), (8, 1))
        per_core.append(dict(idx16=idx16, dstl=dstl, nrmv=nrmv))

    # S_dram layout: per tile, its gathered chunks (cot order) then the diag
    tile_off = np.zeros(n_tiles + 1, np.int64)
    for t in range(n_tiles):
        tile_off[t + 1] = tile_off[t] + len(chunks_of_tile[t]) + 1
    assert tile_off[n_tiles] == nchunk_all

    struct = dict(n_tiles=n_tiles, NR=NR, SL=SL, ranges=ranges, calls=calls,
                  nchunk_g=nchunk_g, nchunk_all=nchunk_all,
                  chunks_of_tile=chunks_of_tile, tile_off=tile_off,
                  max_nb=max(nb for (_, _, nb) in calls) if calls else 1)
    return struct, per_core


# ----------------------------------------------------------------- program --
def _build(struct, n, npc, f_in, f_hid, f_out):
    nt = struct["n_tiles"]
    NR = struct["NR"]
    SL = struct["SL"]
    nchunk_g = struct["nchunk_g"]
    nchunk_all = struct["nchunk_all"]
    tile_off = struct["tile_off"]
    maxb = struct["max_nb"]
    maxcpt = max(len(c) for c in struct["chunks_of_tile"]) + 1
    fdims = [(f_in, f_hid), (f_hid, f_hid), (f_hid, f_out)]
    ic = max(nchunk_g, 1) * 8

    nc = bacc.Bacc("TRN2", target_bir_lowering=False, debug=False,
                   num_devices=N_CORES, num_swdge_queues=NQ)
    xT = nc.dram_tensor("xT", [f_in, npc], fp16, kind="ExternalInput").ap()
    Ws = [nc.dram_tensor(f"W{i+1}", [fi, fo], fp16, kind="ExternalInput").ap()
          for i, (fi, fo) in enumerate(fdims)]
    bs = [nc.dram_tensor(f"b{i+1}", [fo, 1], fp32, kind="ExternalInput").ap()
          for i, (_, fo) in enumerate(fdims)]
    iota_in = nc.dram_tensor("iota", [P, P], fp16, kind="ExternalInput").ap()
    idx_in = nc.dram_tensor("idx_all", [P, ic], i16, kind="ExternalInput").ap()
    dstl_in = nc.dram_tensor("dstl", [P, nchunk_all], fp32, kind="ExternalInput").ap()
    nrm_in = nc.dram_tensor("normv", [P, nchunk_all], fp32, kind="ExternalInput").ap()
    ones_in = nc.dram_tensor("ones40", [f_out, 1], fp32, kind="ExternalInput").ap()
    ones16_in = nc.dram_tensor("ones40h", [f_out, 1], fp16, kind="ExternalInput").ap()
    out3T = nc.dram_tensor("out3T", [f_out, npc], fp32, kind="ExternalOutput").ap()

    with tile.TileContext(nc) as tc:
        with (
            tc.tile_pool(name="const", bufs=1) as cp,
            tc.tile_pool(name="gather", bufs=24) as gp,
            tc.tile_pool(name="sel", bufs=6) as selp,
            tc.tile_pool(name="work", bufs=3) as wp,
            tc.tile_pool(name="persist", bufs=1) as pp,
            tc.tile_pool(name="psA", bufs=4, space="PSUM") as psA,
            tc.tile_pool(name="psB", bufs=2, space="PSUM") as psB,
            tc.tile_pool(name="psC", bufs=1, space="PSUM") as psC,
            tc.tile_pool(name="dram", bufs=1, space="DRAM") as dr,
        ):
            iota_sb = cp.tile([P, P], fp16)
            nc.sync.dma_start(iota_sb[:], iota_in[:])
            idx_sb = cp.tile([P, ic], i16)
            nc.sync.dma_start(idx_sb[:], idx_in[:])
            dstl_sb = cp.tile([P, nchunk_all], fp32)
            nc.sync.dma_start(dstl_sb[:], dstl_in[:])
            nrm_sb = cp.tile([P, nchunk_all], fp32)
            nc.sync.dma_start(nrm_sb[:], nrm_in[:])
            W_sb = []
            b_sb = []
            for i, (fi, fo) in enumerate(fdims):
                w = cp.tile([fi, fo], fp16, tag=f"W{i}")
                nc.sync.dma_start(w[:], Ws[i][:])
                W_sb.append(w)
                b = cp.tile([fo, 1], fp32, tag=f"b{i}")
                nc.sync.dma_start(b[:], bs[i][:])
                b_sb.append(b)
            ones_col = cp.tile([f_out, 1], fp16)
            nc.sync.dma_start(ones_col[:], ones16_in[:])
            ones_row = cp.tile([1, f_out], fp32)
            nc.sync.dma_start(ones_row[:], ones_in[:].transpose([1, 0]))

            xT_sb = pp.tile([f_in, npc], fp16, tag="xT")
            nc.sync.dma_start(xT_sb[:], xT[:])
            yT0 = pp.tile([f_hid, nt * P], fp16, tag="yT0")
            yT1 = pp.tile([f_hid, nt * P], fp16, tag="yT1")
            yT = [yT0, yT1]

            shard_d = dr.tile([npc, 128], fp16)    # 256B rows: local gathers
            shard64_d = dr.tile([npc, 64], fp16)   # compact: collective input
            gfull64_d = dr.tile([n, 64], fp16)     # compact: collective output
            gfull_d = dr.tile([n, 128], fp16)      # 256B rows: slice gathers

            x3e = pp.tile([f_out, nt * P], fp16, tag="yT0")
            g_loc = pp.tile([P, nt, f_hid], fp16, tag="gloc")
            nc.vector.memset(g_loc[:, :, :], 0.0)

            S_dram = dr.tile([P, nchunk_all * P], fp16)

            qrot = [0]
            RWIDE = N_CORES * SL  # rows per slice region of gfull

            def gather_call(cls, lo, nb, fo):
                g_t = gp.tile([P, maxb, fo], fp16, tag="G")
                if cls == 0:
                    in_ap = shard_d[0:npc, 0:fo]
                else:
                    q = cls - 1
                    in_ap = gfull_d[q * RWIDE: (q + 1) * RWIDE, 0:fo]
                dma_gather_raw(
                    nc.gpsimd,
                    out_ap=g_t[:, 0:nb, :],
                    in_ap=in_ap,
                    idxs_ap=idx_sb[:, lo * 8: (lo + nb) * 8],
                    num_idxs=nb * P,
                    elem_size=fo,
                    elem_step=128,
                    queue_num=qrot[0] % NQ,
                )
                qrot[0] += 1
                return g_t

            def h_tile(layer, t):
                """h = y_prev @ W for one node tile; fp16 to g_loc+shard_d."""
                fi, fo = fdims[layer]
                tw = min(P, npc - t * P)
                if layer == 0:
                    lhsT = xT_sb[:, t * P: t * P + tw]
                else:
                    lhsT = yT[(layer + 1) % 2][:fi, t * P: t * P + tw]
                pg = psB.tile([P, fo], fp32, tag="pg", space="PSUM")
                nc.tensor.matmul(pg[:tw, :], lhsT=lhsT, rhs=W_sb[layer][:],
                                 start=True, stop=True)
                gsl = g_loc[:, t, 0:fo]
                nc.vector.tensor_copy(gsl[:tw, :], pg[:tw, :])
                nc.sync.dma_start(shard_d[t * P: t * P + tw, 0:fo],
                                  gsl[:tw, :])
                nc.sync.dma_start(shard64_d[t * P: t * P + tw, 0:fo],
                                  gsl[:tw, :])

            # layer-0 h-phase; later layers' h tiles are emitted inside the
            # previous layer's chain loop (pipelines the layer boundary)
            for t in range(nt):
                h_tile(0, t)

            for layer in range(3):
                fi, fo = fdims[layer]
                # ---- a few local gathers overlap the AllGather (the G pool
                # has limited bufs; issuing more would deadlock the
                # scheduler: their WAR frees sit behind post-AG matmuls) ----
                PRE_AG = 12
                Gt = {}  # (cls, lo) -> tile (keyed by call)
                npre = 0
                for (q, lo, nb) in struct["calls"]:
                    if q == 0 and npre < PRE_AG:
                        Gt[(q, lo)] = gather_call(q, lo, nb, fo)
                        npre += 1

                # ---- sub-AllGathers: sub-AG k ships every core's rows
                # [k*SL, (k+1)*SL) into the contiguous slice-k region of
                # gfull, so slice-k gathers start as soon as it lands ----
                # compact 128B-row collective, then a local restride DMA into
                # the 256B-row gfull the gathers need (halves CC bytes)
                for k in range(NR):
                    nc.gpsimd.collective_compute(
                        "AllGather",
                        mybir.AluOpType.bypass,
                        replica_groups=[list(range(N_CORES))],
                        ins=[shard64_d[k * SL: (k + 1) * SL, :]],
                        outs=[gfull64_d[k * RWIDE: (k + 1) * RWIDE, :]],
                    )
                    nc.sync.dma_start(
                        gfull_d[k * RWIDE: (k + 1) * RWIDE, 0:64],
                        gfull64_d[k * RWIDE: (k + 1) * RWIDE, :])

                # ---- remaining gathers, in range (= consumption) order ----
                for (q, lo, nb) in struct["calls"]:
                    if (q, lo) not in Gt:
                        Gt[(q, lo)] = gather_call(q, lo, nb, fo)

                # map global chunk id -> (tile handle, block within call)
                chunk_tile = {}
                for (q, lo, nb) in struct["calls"]:
                    for b in range(nb):
                        chunk_tile[lo + b] = (Gt[(q, lo)], b)

                # ---- per-tile accumulation chains ----
                for t in range(nt):
                    tw = min(P, npc - t * P)
                    cot = struct["chunks_of_tile"][t]
                    ncot = len(cot)
                    wS = (ncot + 1) * P
                    S_tbuf = selp.tile([P, maxcpt * P], fp16, tag="St")
                    if layer == 0:
                        # build the tile's S stack once; reused by layers 1-2
                        for j, (g, q) in enumerate(cot):
                            nc.vector.tensor_scalar(
                                out=S_tbuf[:, j * P: (j + 1) * P],
                                in0=iota_sb[:],
                                scalar1=dstl_sb[:, g: g + 1],
                                scalar2=nrm_sb[:, g: g + 1],
                                op0=mybir.AluOpType.is_equal,
                                op1=mybir.AluOpType.mult,
                            )
                        gd = nchunk_g + t
                        nc.vector.tensor_scalar(
                            out=S_tbuf[:, ncot * P: wS],
                            in0=iota_sb[:],
                            scalar1=dstl_sb[:, gd: gd + 1],
                            scalar2=nrm_sb[:, gd: gd + 1],
                            op0=mybir.AluOpType.is_equal,
                            op1=mybir.AluOpType.mult,
                        )
                        nc.sync.dma_start(
                            S_dram[:, tile_off[t] * P: tile_off[t + 1] * P],
                            S_tbuf[:, 0:wS])
                    else:
                        nc.sync.dma_start(
                            S_tbuf[:, 0:wS],
                            S_dram[:, tile_off[t] * P: tile_off[t + 1] * P])
                    pa = psA.tile([fo, P], fp32, tag="pa", space="PSUM")
                    for j, (g, q) in enumerate(cot):
                        g_t, blk = chunk_tile[g]
                        nc.tensor.matmul(
                            pa[:, :],
                            lhsT=g_t[:, blk, :],
                            rhs=S_tbuf[:, j * P: (j + 1) * P],
                            start=(j == 0),
                            stop=False,
                        )
                    # diag (self-loop) chunk: lhsT = local activations
                    nc.tensor.matmul(
                        pa[:, :],
                        lhsT=g_loc[:, t, 0:fo],
                        rhs=S_tbuf[:, ncot * P: wS],
                        start=False,
                        stop=True,
                    )
                    if layer < 2:
                        nc.scalar.activation(
                            out=yT[layer % 2][:fo, t * P: t * P + tw],
                            in_=pa[:, :tw],
                            func=mybir.ActivationFunctionType.Relu,
                            bias=b_sb[layer][:, :1],
                            scale=1.0,
                        )
                        # next layer's h for this tile, right behind the
                        # epilogue: shard_d fills as chains drain, so the
                        # next sub-AGs can start with no serial h-phase
                        h_tile(layer + 1, t)
                    else:
                        nc.scalar.activation(
                            out=x3e[:, t * P: t * P + tw],
                            in_=pa[:, :tw],
                            func=mybir.ActivationFunctionType.Exp,
                            bias=b_sb[2][:, :1],
                            scale=1.0,
                        )

            # ---- log_softmax tail: out = ln(e) - ln(sum_part(e)) ----
            W3T = 512
            for o in range(0, npc, W3T):
                wdt = min(W3T, npc - o)
                ps_s = psC.tile([1, W3T], fp32, tag="l3s", space="PSUM")
                nc.tensor.matmul(ps_s[:1, :wdt], lhsT=ones_col[:],
                                 rhs=x3e[:, o: o + wdt], start=True, stop=True)
                ls_t = wp.tile([1, W3T], fp32, tag="ls")
                nc.scalar.activation(
                    out=ls_t[:1, :wdt], in_=ps_s[:1, :wdt],
                    func=mybir.ActivationFunctionType.Ln, bias=0.0, scale=1.0,
                )
                nc.scalar.activation(
                    out=x3e[:, o: o + wdt], in_=x3e[:, o: o + wdt],
                    func=mybir.ActivationFunctionType.Ln, bias=0.0, scale=1.0,
                )
                ps_b = psC.tile([f_out, W3T], fp32, tag="l3b", space="PSUM")
                nc.tensor.matmul(ps_b[:, :wdt], lhsT=ones_row[:],
                                 rhs=ls_t[:1, :wdt], start=True, stop=True)
                o_sb = wp.tile([f_out, W3T], fp32, tag="o3")
                nc.vector.tensor_tensor(
                    out=o_sb[:, :wdt], in0=x3e[:, o: o + wdt],
                    in1=ps_b[:, :wdt], op=mybir.AluOpType.subtract,
                )
                nc.sync.dma_start(out3T[:, o: o + wdt], o_sb[:, :wdt])

    nc.compile()
    return nc


# ----------------------------------------------------------------- kernel ---
_CACHE = {}


def kernel(x, edge_index, W1, b1, W2, b2, W3, b3):
    global LAST_RESULT
    x = np.asarray(x)
    edge_index = np.asarray(edge_index)
    n, f_in = x.shape
    f_hid = np.asarray(W2).shape[0]
    f_out = np.asarray(W3).shape[1]
    assert n % N_CORES == 0
    npc = n // N_CORES

    pkey = (edge_index.shape, int(edge_index[0, 0]), int(edge_index[1, -1]),
            int(edge_index[0].sum() % (1 << 62)))
    hit = _CACHE.get(("prep", pkey))
    if hit is None:
        hit = _prepare_spmd(edge_index, n, npc)
        _CACHE[("prep", pkey)] = hit
    struct, per_core = hit

    ckey = (n, f_in, f_hid, f_out, struct["nchunk_g"], struct["max_nb"],
            tuple(struct["ranges"]))
    if ckey not in _CACHE:
        _CACHE[ckey] = _build(struct, n, npc, f_in, f_hid, f_out)
    nc = _CACHE[ckey]

    iota = np.broadcast_to(np.arange(P, dtype=np.float16), (P, P)).copy()
    ones40 = np.ones((f_out, 1), np.float32)
    in_maps = []
    for c in range(N_CORES):
        pc = per_core[c]
        in_maps.append({
            "xT": np.ascontiguousarray(x[c * npc: (c + 1) * npc].T).astype(np.float16),
            "W1": np.asarray(W1, np.float16), "b1": np.asarray(b1, np.float32).reshape(-1, 1),
            "W2": np.asarray(W2, np.float16), "b2": np.asarray(b2, np.float32).reshape(-1, 1),
            "W3": np.asarray(W3, np.float16),
            # -8 shift: log_softmax is shift-invariant; keeps fp16 exp in range
            "b3": np.asarray(b3, np.float32).reshape(-1, 1) - 8.0,
            "iota": iota, "idx_all": pc["idx16"], "dstl": pc["dstl"],
            "normv": pc["nrmv"],
            "ones40": ones40, "ones40h": ones40.astype(np.float16),
        })
    kw = {}
    if TRACE:
        import tempfile
        kw = dict(trace=True, trace_cores=[0],
                  tmpdir=tempfile.mkdtemp(prefix="gcn_v2_"))
    res = run_bass_kernel_spmd(nc, in_maps, core_ids=list(range(N_CORES)), **kw)
    LAST_RESULT = res
    out = np.concatenate(
        [res.results[c]["out3T"].T for c in range(N_CORES)], axis=0
    ).astype(np.float32)
    return out


# revision 32
# speedup vs baseline: 15.9178x; 15.9178x over previous
"""GCN (3-layer, PyG GCNConv semantics) on 8 Trainium2 NeuronCores.

Strategy (graph/data parallel, dst-sharded), v2:
  - Nodes sharded across 8 cores (rows of x / output).
  - Per layer: each core computes its slice of h = y_prev @ W on PE,
    writes it (fp16, 256B-strided rows) to DRAM, AllGather -> full g table.
  - Aggregation: edges bucketed by (dst tile, src class) where class is
    "local" (src in own shard, gathered from the pre-collective shard_d,
    overlapping the AllGather) or one of ceil(N/32768) regions of gfull
    (int16 gather indices are region-relative).  Chunks of 128 edges;
    dma_gather (custom 128B-payload lowering) fetches g[src] rows across
    4 concurrent SWDGE queues (num_swdge_queues=4, queue_num rotation —
    ~2.2 ns/row vs 8 single-queue); a norm-valued one-hot S
    ([128 edges x 128 dst], ONE dual-op tensor_scalar: (iota==dst)*norm)
    turns the scatter-add into PE matmuls accumulated in PSUM:
    agg_T = G.T @ S.  Self-loops are a diag chunk (dst=iota, norm=invdeg)
    whose lhsT is the local activation tile (no gather).
  - Epilogue: relu(agg + b) in one ScalarE activation (transposed layout:
    bias is per-partition). Final layer: log_softmax via exp (ACT),
    partition-sum (PE ones-matmul), ln (ACT), broadcast (PE), subtract.
  - Output is produced transposed [40, nodes] per core; host transposes.

Self-contained: only needs numpy + the concourse stack at /opt/trn_rl_repo.
"""

import os
import sys

sys.path.insert(0, "/opt/trn_rl_repo")

import numpy as np

import concourse.bacc as bacc
import concourse.tile as tile
import concourse.mybir as mybir
from concourse import ap_utils
from concourse.bass import AP, MemorySpace
from concourse.bass_utils import run_bass_kernel_spmd

fp32 = mybir.dt.float32
fp16 = mybir.dt.float16
i16 = mybir.dt.int16

N_CORES = 8
P = 128
NBMAX = 8          # gather blocks per call (single-packet limit: 1024 rows)
SL_CAP = 4095      # max slice rows: N_CORES*SL must fit int16 gather index
NQ = 4             # SWDGE queues

# test.py sets this to capture profile info; harness leaves it off.
TRACE = os.environ.get("GCN_TRACE", "0") == "1"
# strided (64-col) collective APs are rejected by walrus; keep full rows
AG64 = os.environ.get("GCN_AG64", "0") == "1"
LAST_RESULT = None


# ---------------------------------------------------------------- gather ----
def dma_gather_raw(engine, out_ap, in_ap, idxs_ap, num_idxs, elem_size, elem_step,
                   queue_num=0, single_packet=True):
    """bass dma_gather minus the elem_size%256B assert: the ucode only needs
    the row STRIDE 256B-quantized; the payload is free."""
    assert idxs_ap.dtype == mybir.dt.int16
    assert in_ap.space == MemorySpace.DRAM
    assert out_ap.space == MemorySpace.SBUF
    assert in_ap.dtype == out_ap.dtype
    assert ap_utils.ap_is_contiguous(out_ap.ap[1:])
    assert ap_utils.ap_is_contiguous(idxs_ap.ap[1:])
    assert in_ap.ap[-1][1] == elem_size
    assert out_ap.ap[-1][1] == elem_size
    assert in_ap.ap[0][0] == elem_step
    stride_bytes = elem_step * mybir.dt.size(in_ap.dtype)
    assert stride_bytes % 256 == 0
    return engine.add_instruction(
        mybir.InstDMAGatherAnt(
            name=engine.bass.get_next_instruction_name(),
            ins=[
                *engine.lower_ap_dma(in_ap, for_custom_bir_dma=True),
                engine.lower_ap(idxs_ap),
                engine.lower_val_access(engine.to_reg(num_idxs)),
            ],
            outs=[engine.lower_ap(out_ap)],
            transpose=False,
            num_idxs=num_idxs,
            elem_size=elem_size,
            stride_bytes_256=stride_bytes // 256,
            gen_mode=0,
            single_packet=single_packet,
            queue_num=queue_num,
            sbuf_tokens_per_rank=0,
            sbuf_free_dim_per_rank=0,
            sbuf_free_dim_pad_per_rank=0,
            sbuf_byte_offset=0,
        )
    )


# ---------------------------------------------------------- host preprocess --
def _prepare_spmd(edge_index, n, npc):
    """Shared (max-over-cores) chunk layout so all cores run one program.

    Src classes: 0 = local (own shard, gathered from shard_d pre-collective),
    1+k = slice k.  The AllGather is split into NR sub-collectives; sub-AG k
    gathers every core's local rows [k*SL, (k+1)*SL) into a contiguous
    gfull region of N_CORES*SL rows ordered (core, row%SL), so slice-k
    gathers can start as soon as sub-AG k lands.
    """
    src = np.asarray(edge_index[0], np.int64)
    dst = np.asarray(edge_index[1], np.int64)
    deg = np.bincount(dst, minlength=n).astype(np.float64) + 1.0
    dis = 1.0 / np.sqrt(deg)
    norm_all = (dis[src] * dis[dst]).astype(np.float32)
    invdeg = (1.0 / deg).astype(np.float32)

    NR = (npc + SL_CAP - 1) // SL_CAP
    while npc % NR != 0 and NR < npc:
        NR += 1
    SL = npc // NR
    n_tiles = (npc + P - 1) // P
    NCLS = 1 + NR

    # per-core bucket counts
    cnts = np.zeros((N_CORES, n_tiles, NCLS), np.int64)
    core_edges = []
    for c in range(N_CORES):
        m = (dst >= c * npc) & (dst < (c + 1) * npc)
        s_c, d_c, nrm_c = src[m], dst[m] - c * npc, norm_all[m]
        tile_c = d_c // P
        own = (s_c >= c * npc) & (s_c < (c + 1) * npc)
        src_core = s_c // npc
        src_loc = s_c - src_core * npc
        slc = src_loc // SL
        cls = np.where(own, 0, 1 + slc)
        rel = np.where(own, src_loc,
                       src_core * SL + (src_loc - slc * SL))
        order = np.lexsort((s_c, cls, tile_c))
        tile_s, cls_s = tile_c[order], cls[order]
        bucket = tile_s * NCLS + cls_s
        cnts[c] = np.bincount(bucket, minlength=n_tiles * NCLS).reshape(
            n_tiles, NCLS)
        core_edges.append((bucket, rel[order], (d_c[order] - tile_s * P),
                           nrm_c[order]))

    kc = (cnts.max(axis=0) + P - 1) // P  # shared chunks per (tile, cls)

    # ranges are for slice classes; local chunks are packed globally below
    ranges = []
    start = 0
    while start < n_tiles:
        end = start + 1
        while end < n_tiles:
            if kc[start:end + 1, 1:].sum(axis=0).max() > NBMAX:
                break
            end += 1
        ranges.append((start, end))
        start = end

    chunk_base = np.zeros((n_tiles, NCLS), np.int64)
    calls = []
    g = 0
    # local (cls 0) chunks first, tile order, full-size calls: they all
    # overlap the AllGather, so fewer/bigger calls matter here
    for t in range(n_tiles):
        chunk_base[t, 0] = g
        g += int(kc[t, 0])
    pos = 0
    while pos < g:
        nb = min(NBMAX, g - pos)
        calls.append((0, pos, nb))
        pos += nb
    n_local_calls = len(calls)
    for (t0, t1) in ranges:
        for q in range(1, NCLS):
            nb = int(kc[t0:t1, q].sum())
            if nb == 0:
                continue
            lo = g
            for t in range(t0, t1):
                chunk_base[t, q] = g
                g += int(kc[t, q])
            # split oversized calls (single tile can exceed NBMAX chunks):
            # the 1024-row single-packet limit caps nb at 8 per call
            while nb > NBMAX:
                calls.append((q, lo, NBMAX))
                lo += NBMAX
                nb -= NBMAX
            calls.append((q, lo, nb))
    nchunk_g = g
    chunks_of_tile = []
    for t in range(n_tiles):
        lst = []
        for q in range(NCLS):
            for k in range(int(kc[t, q])):
                lst.append((int(chunk_base[t, q]) + k, q))
        chunks_of_tile.append(lst)

    nchunk_all = nchunk_g + n_tiles
    per_core = []
    for c in range(N_CORES):
        bucket, rel_s, dl_s, nrm_s = core_edges[c]
        cnt = cnts[c].reshape(-1)
        bs = np.concatenate([[0], np.cumsum(cnt)[:-1]])
        rank = np.arange(len(bucket)) - bs[bucket]
        t_of = bucket // NCLS
        q_of = bucket % NCLS
        gchunk = chunk_base[t_of, q_of] + rank // P
        slot = rank % P

        idx_flat = np.zeros(max(nchunk_g, 1) * P, np.int64)
        idx_flat[gchunk * P + slot] = rel_s
        dstl = np.zeros((P, nchunk_all), np.float32)
        nrmv = np.zeros((P, nchunk_all), np.float32)
        dstl[slot, gchunk] = dl_s
        nrmv[slot, gchunk] = nrm_s
        node = c * npc + np.arange(npc)
        dstl[np.arange(npc) % P, nchunk_g + np.arange(npc) // P] = \
            np.arange(npc) % P
        nrmv[np.arange(npc) % P, nchunk_g + np.arange(npc) // P] = invdeg[node]

        tmp = idx_flat.astype(np.int16).reshape(max(nchunk_g, 1) * 8, 16).T
        idx16 = np.tile(np.ascontiguousarray(tmp), (8, 1))
        per_core.append(dict(idx16=idx16, dstl=dstl, nrmv=nrmv))

    # S_dram layout: per tile, its gathered chunks (cot order) then the diag
    tile_off = np.zeros(n_tiles + 1, np.int64)
    for t in range(n_tiles):
        tile_off[t + 1] = tile_off[t] + len(chunks_of_tile[t]) + 1
    assert tile_off[n_tiles] == nchunk_all

    struct = dict(n_tiles=n_tiles, NR=NR, SL=SL, ranges=ranges, calls=calls,
                  nchunk_g=nchunk_g, nchunk_all=nchunk_all,
                  chunks_of_tile=chunks_of_tile, tile_off=tile_off,
                  n_local_calls=n_local_calls,
                  max_nb=max(nb for (_, _, nb) in calls) if calls else 1)
    return struct, per_core


# ----------------------------------------------------------------- program --
def _build(struct, n, npc, f_in, f_hid, f_out):
    nt = struct["n_tiles"]
    NR = struct["NR"]
    SL = struct["SL"]
    nchunk_g = struct["nchunk_g"]
    nchunk_all = struct["nchunk_all"]
    tile_off = struct["tile_off"]
    maxb = struct["max_nb"]
    maxcpt = max(len(c) for c in struct["chunks_of_tile"]) + 1
    fdims = [(f_in, f_hid), (f_hid, f_hid), (f_hid, f_out)]
    ic = max(nchunk_g, 1) * 8

    nc = bacc.Bacc("TRN2", target_bir_lowering=False, debug=False,
                   num_devices=N_CORES, num_swdge_queues=NQ)
    xT = nc.dram_tensor("xT", [f_in, npc], fp16, kind="ExternalInput").ap()
    Ws = [nc.dram_tensor(f"W{i+1}", [fi, fo], fp16, kind="ExternalInput").ap()
          for i, (fi, fo) in enumerate(fdims)]
    bs = [nc.dram_tensor(f"b{i+1}", [fo, 1], fp32, kind="ExternalInput").ap()
          for i, (_, fo) in enumerate(fdims)]
    iota_in = nc.dram_tensor("iota", [P, P], fp16, kind="ExternalInput").ap()
    idx_in = nc.dram_tensor("idx_all", [P, ic], i16, kind="ExternalInput").ap()
    dstl_in = nc.dram_tensor("dstl", [P, nchunk_all], fp32, kind="ExternalInput").ap()
    nrm_in = nc.dram_tensor("normv", [P, nchunk_all], fp32, kind="ExternalInput").ap()
    ones_in = nc.dram_tensor("ones40", [f_out, 1], fp32, kind="ExternalInput").ap()
    ones16_in = nc.dram_tensor("ones40h", [f_out, 1], fp16, kind="ExternalInput").ap()
    out3T = nc.dram_tensor("out3T", [f_out, npc], fp32, kind="ExternalOutput").ap()

    with tile.TileContext(nc) as tc:
        with (
            tc.tile_pool(name="const", bufs=1) as cp,
            tc.tile_pool(name="gather", bufs=min(40, struct["n_local_calls"] + 10)) as gp,
            tc.tile_pool(name="sel", bufs=6) as selp,
            tc.tile_pool(name="work", bufs=3) as wp,
            tc.tile_pool(name="persist", bufs=1) as pp,
            tc.tile_pool(name="psA", bufs=4, space="PSUM") as psA,
            tc.tile_pool(name="psB", bufs=2, space="PSUM") as psB,
            tc.tile_pool(name="psC", bufs=1, space="PSUM") as psC,
            tc.tile_pool(name="dram", bufs=1, space="DRAM") as dr,
        ):
            iota_sb = cp.tile([P, P], fp16)
            nc.sync.dma_start(iota_sb[:], iota_in[:])
            idx_sb = cp.tile([P, ic], i16)
            nc.sync.dma_start(idx_sb[:], idx_in[:])
            dstl_sb = cp.tile([P, nchunk_all], fp32)
            nc.sync.dma_start(dstl_sb[:], dstl_in[:])
            nrm_sb = cp.tile([P, nchunk_all], fp32)
            nc.sync.dma_start(nrm_sb[:], nrm_in[:])
            W_sb = []
            b_sb = []
            for i, (fi, fo) in enumerate(fdims):
                w = cp.tile([fi, fo], fp16, tag=f"W{i}")
                nc.sync.dma_start(w[:], Ws[i][:])
                W_sb.append(w)
                b = cp.tile([fo, 1], fp32, tag=f"b{i}")
                nc.sync.dma_start(b[:], bs[i][:])
                b_sb.append(b)
            ones_col = cp.tile([f_out, 1], fp16)
            nc.sync.dma_start(ones_col[:], ones16_in[:])
            ones_row = cp.tile([1, f_out], fp32)
            nc.sync.dma_start(ones_row[:], ones_in[:].transpose([1, 0]))

            xT_sb = pp.tile([f_in, npc], fp16, tag="xT")
            nc.sync.dma_start(xT_sb[:], xT[:])
            yT0 = pp.tile([f_hid, nt * P], fp16, tag="yT0")
            yT1 = pp.tile([f_hid, nt * P], fp16, tag="yT1")
            yT = [yT0, yT1]

            shard_d = dr.tile([npc, 128], fp16)    # 256B rows: local gathers
            gfull_d = dr.tile([n, 128], fp16)      # 256B rows: slice gathers

            x3e = pp.tile([f_out, nt * P], fp16, tag="yT0")
            g_loc = pp.tile([P, nt, f_hid], fp16, tag="gloc")
            nc.vector.memset(g_loc[:, :, :], 0.0)

            S_dram = dr.tile([P, nchunk_all * P], fp16)

            qload = [0] * NQ
            RWIDE = N_CORES * SL  # rows per slice region of gfull

            def gather_call(cls, lo, nb, fo):
                g_t = gp.tile([P, maxb, fo], fp16, tag="G")
                if cls == 0:
                    in_ap = shard_d[0:npc, 0:fo]
                else:
                    q = cls - 1
                    in_ap = gfull_d[q * RWIDE: (q + 1) * RWIDE, 0:fo]
                qn = min(range(NQ), key=lambda i: qload[i])
                qload[qn] += nb
                dma_gather_raw(
                    nc.gpsimd,
                    out_ap=g_t[:, 0:nb, :],
                    in_ap=in_ap,
                    idxs_ap=idx_sb[:, lo * 8: (lo + nb) * 8],
                    num_idxs=nb * P,
                    elem_size=fo,
                    elem_step=128,
                    queue_num=qn,
                )
                return g_t

            def h_tile(layer, t):
                """h = y_prev @ W for one node tile; fp16 to g_loc+shard_d."""
                fi, fo = fdims[layer]
                tw = min(P, npc - t * P)
                if layer == 0:
                    lhsT = xT_sb[:, t * P: t * P + tw]
                else:
                    lhsT = yT[(layer + 1) % 2][:fi, t * P: t * P + tw]
                pg = psB.tile([P, fo], fp32, tag="pg", space="PSUM")
                nc.tensor.matmul(pg[:tw, :], lhsT=lhsT, rhs=W_sb[layer][:],
                                 start=True, stop=True)
                gsl = g_loc[:, t, 0:fo]
                nc.vector.tensor_copy(gsl[:tw, :], pg[:tw, :])
                nc.sync.dma_start(shard_d[t * P: t * P + tw, 0:fo],
                                  gsl[:tw, :])

            # layer-0 h-phase; later layers' h tiles are emitted inside the
            # previous layer's chain loop (pipelines the layer boundary)
            for t in range(nt):
                h_tile(0, t)

            for layer in range(3):
                fi, fo = fdims[layer]
                # ---- all local-class gathers overlap the AllGather (G pool
                # must have enough bufs to hold them plus a slice lookahead,
                # else the scheduler deadlocks on pool WAR) ----
                Gt = {}  # (cls, lo) -> tile (keyed by call)
                for (q, lo, nb) in struct["calls"]:
                    if q == 0:
                        Gt[(q, lo)] = gather_call(q, lo, nb, fo)

                # ---- sub-AllGathers: sub-AG k ships every core's rows
                # [k*SL, (k+1)*SL) into the contiguous slice-k region of
                # gfull, so slice-k gathers start as soon as it lands ----
                for k in range(NR):
                    nc.gpsimd.collective_compute(
                        "AllGather",
                        mybir.AluOpType.bypass,
                        replica_groups=[list(range(N_CORES))],
                        ins=[shard_d[k * SL: (k + 1) * SL, :]],
                        outs=[gfull_d[k * RWIDE: (k + 1) * RWIDE, :]],
                    )

                # ---- remaining gathers, in range (= consumption) order ----
                for (q, lo, nb) in struct["calls"]:
                    if (q, lo) not in Gt:
                        Gt[(q, lo)] = gather_call(q, lo, nb, fo)

                # map global chunk id -> (tile handle, block within call)
                chunk_tile = {}
                for (q, lo, nb) in struct["calls"]:
                    for b in range(nb):
                        chunk_tile[lo + b] = (Gt[(q, lo)], b)

                # ---- per-tile accumulation chains ----
                for t in range(nt):
                    tw = min(P, npc - t * P)
                    cot = struct["chunks_of_tile"][t]
                    ncot = len(cot)
                    wS = (ncot + 1) * P
                    S_tbuf = selp.tile([P, maxcpt * P], fp16, tag="St")
                    if layer == 0:
                        # build the tile's S stack once; reused by layers 1-2
                        for j, (g, q) in enumerate(cot):
                            nc.vector.tensor_scalar(
                                out=S_tbuf[:, j * P: (j + 1) * P],
                                in0=iota_sb[:],
                                scalar1=dstl_sb[:, g: g + 1],
                                scalar2=nrm_sb[:, g: g + 1],
                                op0=mybir.AluOpType.is_equal,
                                op1=mybir.AluOpType.mult,
                            )
                        gd = nchunk_g + t
                        nc.vector.tensor_scalar(
                            out=S_tbuf[:, ncot * P: wS],
                            in0=iota_sb[:],
                            scalar1=dstl_sb[:, gd: gd + 1],
                            scalar2=nrm_sb[:, gd: gd + 1],
                            op0=mybir.AluOpType.is_equal,
                            op1=mybir.AluOpType.mult,
                        )
                        nc.sync.dma_start(
                            S_dram[:, tile_off[t] * P: tile_off[t + 1] * P],
                            S_tbuf[:, 0:wS])
                    else:
                        nc.sync.dma_start(
                            S_tbuf[:, 0:wS],
                            S_dram[:, tile_off[t] * P: tile_off[t + 1] * P])
                    pa = psA.tile([fo, P], fp32, tag="pa", space="PSUM")
                    for j, (g, q) in enumerate(cot):
                        g_t, blk = chunk_tile[g]
                        nc.tensor.matmul(
                            pa[:, :],
                            lhsT=g_t[:, blk, :],
                            rhs=S_tbuf[:, j * P: (j + 1) * P],
                            start=(j == 0),
                            stop=False,
                        )
                    # diag (self-loop) chunk: lhsT = local activations
                    nc.tensor.matmul(
                        pa[:, :],
                        lhsT=g_loc[:, t, 0:fo],
                        rhs=S_tbuf[:, ncot * P: wS],
                        start=False,
                        stop=True,
                    )
                    if layer < 2:
                        nc.scalar.activation(
                            out=yT[layer % 2][:fo, t * P: t * P + tw],
                            in_=pa[:, :tw],
                            func=mybir.ActivationFunctionType.Relu,
                            bias=b_sb[layer][:, :1],
                            scale=1.0,
                        )
                        # next layer's h for this tile, right behind the
                        # epilogue: shard_d fills as chains drain, so the
                        # next sub-AGs can start with no serial h-phase
                        h_tile(layer + 1, t)
                    else:
                        nc.scalar.activation(
                            out=x3e[:, t * P: t * P + tw],
                            in_=pa[:, :tw],
                            func=mybir.ActivationFunctionType.Exp,
                            bias=b_sb[2][:, :1],
                            scale=1.0,
                        )

            # ---- log_softmax tail: out = ln(e) - ln(sum_part(e)) ----
            W3T = 512
            for o in range(0, npc, W3T):
                wdt = min(W3T, npc - o)
                ps_s = psC.tile([1, W3T], fp32, tag="l3s", space="PSUM")
                nc.tensor.matmul(ps_s[:1, :wdt], lhsT=ones_col[:],
                                 rhs=x3e[:, o: o + wdt], start=True, stop=True)
                ls_t = wp.tile([1, W3T], fp32, tag="ls")
                nc.scalar.activation(
                    out=ls_t[:1, :wdt], in_=ps_s[:1, :wdt],
                    func=mybir.ActivationFunctionType.Ln, bias=0.0, scale=1.0,
                )
                nc.scalar.activation(
                    out=x3e[:, o: o + wdt], in_=x3e[:, o: o + wdt],
                    func=mybir.ActivationFunctionType.Ln, bias=0.0, scale=1.0,
                )
                ps_b = psC.tile([f_out, W3T], fp32, tag="l3b", space="PSUM")
                nc.tensor.matmul(ps_b[:, :wdt], lhsT=ones_row[:],
                                 rhs=ls_t[:1, :wdt], start=True, stop=True)
                o_sb = wp.tile([f_out, W3T], fp32, tag="o3")
                nc.vector.tensor_tensor(
                    out=o_sb[:, :wdt], in0=x3e[:, o: o + wdt],
                    in1=ps_b[:, :wdt], op=mybir.AluOpType.subtract,
                )
                nc.sync.dma_start(out3T[:, o: o + wdt], o_sb[:, :wdt])

    nc.compile()
    return nc


# ----------------------------------------------------------------- kernel ---
_CACHE = {}


def kernel(x, edge_index, W1, b1, W2, b2, W3, b3):
    global LAST_RESULT
    x = np.asarray(x)
    edge_index = np.asarray(edge_index)
    n, f_in = x.shape
    f_hid = np.asarray(W2).shape[0]
    f_out = np.asarray(W3).shape[1]
    assert n % N_CORES == 0
    npc = n // N_CORES

    pkey = (edge_index.shape, int(edge_index[0, 0]), int(edge_index[1, -1]),
            int(edge_index[0].sum() % (1 << 62)))
    hit = _CACHE.get(("prep", pkey))
    if hit is None:
        hit = _prepare_spmd(edge_index, n, npc)
        _CACHE[("prep", pkey)] = hit
    struct, per_core = hit

    ckey = (n, f_in, f_hid, f_out, struct["nchunk_g"], struct["max_nb"],
            tuple(struct["ranges"]))
    if ckey not in _CACHE:
        _CACHE[ckey] = _build(struct, n, npc, f_in, f_hid, f_out)
    nc = _CACHE[ckey]

    iota = np.broadcast_to(np.arange(P, dtype=np.float16), (P, P)).copy()
    ones40 = np.ones((f_out, 1), np.float32)
    in_maps = []
    for c in range(N_CORES):
        pc = per_core[c]
        in_maps.append({
            "xT": np.ascontiguousarray(x[c * npc: (c + 1) * npc].T).astype(np.float16),
            "W1": np.asarray(W1, np.float16), "b1": np.asarray(b1, np.float32).reshape(-1, 1),
            "W2": np.asarray(W2, np.float16), "b2": np.asarray(b2, np.float32).reshape(-1, 1),
            "W3": np.asarray(W3, np.float16),
            # -8 shift: log_softmax is shift-invariant; keeps fp16 exp in range
            "b3": np.asarray(b3, np.float32).reshape(-1, 1) - 8.0,
            "iota": iota, "idx_all": pc["idx16"], "dstl": pc["dstl"],
            "normv": pc["nrmv"],
            "ones40": ones40, "ones40h": ones40.astype(np.float16),
        })
    kw = {}
    if TRACE:
        import tempfile
        kw = dict(trace=True, trace_cores=[0],
                  tmpdir=tempfile.mkdtemp(prefix="gcn_v2_"))
    res = run_bass_kernel_spmd(nc, in_maps, core_ids=list(range(N_CORES)), **kw)
    LAST_RESULT = res
    out = np.concatenate(
        [res.results[c]["out3T"].T for c in range(N_CORES)], axis=0
    ).astype(np.float32)
    return out
